# revision 12
# baseline (speedup 1.0000x reference)
"""Trainium2 Bass kernel for nn_MultiHeadAttention (B=4, S=2048, D=1024, H=16, HD=64).

Sharding: 8 cores = 4 batches (data parallel) x 2 head-groups of 8 heads
(tensor parallel). Each core computes its batch's QKV projections for its 8
heads, full softmax attention, and the partial output projection for its head
group. The host sums the two head-group partials per batch (the hinted
all-reduce, done at gather time) and adds the output bias.

v2 vs baseline: all matmul operands in bf16 (the fp32r sweep tripped the HAM
power throttle to a 50% duty cycle; bf16 also halves DMA+SBUF), approximate
reciprocal for the softmax denominator (exact DVE reciprocal was 3.3us per
call and gated each unit), and a pair-outer sweep with the Q/K projections of
pair p+1 interleaved between attention units of pair p so the ACT engine
(which carries all 33.5M exp elements -- the ~255us floor) starts ~40us
earlier and never starves.

Per-core layout (all matmuls bf16, N=512 -> 1 cycle/row):
  - x^T tiles resident in SBUF; Q^T/K^T [dh, tok] per head pair, scores
    transposed S^T[k, q] via PE row packing (tile_position (0,0)/(64,0), K=64).
  - exp on ACT from 2-bank PSUM [128, 1024] into E^T bf16 tiles (scale folded).
  - PV: lhsT = V' [128, 65] with a ones column collecting the denominator Z.
  - normalize: DVE reciprocal_approx_fast on Z + gpsimd partition_broadcast +
    DVE multiply -> O^T bf16; out-proj per qc after pair 3's unit.
PSUM budget: pp 2x[128,512] + pssc 2x[128,1024] + pso 2x[65,512] = 8 banks.
"""

import numpy as np
import ml_dtypes
from contextlib import ExitStack

B, S, D = 4, 2048, 1024
H, HD = 16, 64
NCORES = 8
HPC = H // 2            # heads per core = 8
PAIRS = HPC // 2        # head pairs per core = 4
DH = HPC * HD           # per-core head dims = 512
P = 128
TOK_T = S // P          # 16 token tiles of 128
QCC = S // 512          # 4 query chunks of 512
QW = 512
KC = S // P             # 16 key chunks of 128
FC = D // P             # 8 feature chunks of 128
KPG = 2                 # key tiles per exp group
NG = KC // KPG          # 8 exp groups per (pair, qc)

_CACHE = {}


def _build():
    import concourse.bacc as bacc
    import concourse.mybir as mybir
    import concourse.tile as tile

    dt = mybir.dt
    f32 = dt.float32
    bf16 = dt.bfloat16
    AF = mybir.ActivationFunctionType

    nc = bacc.Bacc("TRN2", target_bir_lowering=False, debug=False)

    xqT = nc.dram_tensor("xqT", [D, S], bf16, kind="ExternalInput")
    xkT = nc.dram_tensor("xkT", [D, S], bf16, kind="ExternalInput")
    xvT = nc.dram_tensor("xvT", [D, S], bf16, kind="ExternalInput")
    wq = nc.dram_tensor("wq", [D, DH], bf16, kind="ExternalInput")
    wk = nc.dram_tensor("wk", [D, DH], bf16, kind="ExternalInput")
    wv = nc.dram_tensor("wv", [D, DH], bf16, kind="ExternalInput")
    wo = nc.dram_tensor("wo", [DH, D], bf16, kind="ExternalInput")
    biases = nc.dram_tensor("biases", [P, 3 * PAIRS], f32, kind="ExternalInput")
    out = nc.dram_tensor("out", [S, D], f32, kind="ExternalOutput")

    SCALE = 1.0 / float(np.sqrt(HD))

    def mmr(psum, lhsT, rhs, **kw):
        nc.tensor.matmul(psum, lhsT, rhs, **kw)

    with tile.TileContext(nc, pool_alloc_mode="queue") as tc, ExitStack() as ctx:
        # ---- pools ----
        xq_pool = ctx.enter_context(tc.tile_pool(name="xq", bufs=FC))
        xk_pool = ctx.enter_context(tc.tile_pool(name="xk", bufs=FC))
        wqk_pool = ctx.enter_context(tc.tile_pool(name="wqk", bufs=2 * FC))
        wo_pool = ctx.enter_context(tc.tile_pool(name="wop", bufs=2 * PAIRS))
        qt_pool = ctx.enter_context(tc.tile_pool(name="qt", bufs=PAIRS))
        kt_pool = ctx.enter_context(tc.tile_pool(name="kt", bufs=PAIRS))
        vpr_pool = ctx.enter_context(tc.tile_pool(name="vpr", bufs=TOK_T))
        ot_pool = ctx.enter_context(tc.tile_pool(name="ot", bufs=16))
        zr_pool = ctx.enter_context(tc.tile_pool(name="zr", bufs=2))
        zb_pool = ctx.enter_context(tc.tile_pool(name="zb", bufs=2))
        os_pool = ctx.enter_context(tc.tile_pool(name="os", bufs=2))
        bias_pool = ctx.enter_context(tc.tile_pool(name="bias", bufs=1))

        # ---- DMA: spread input loads across engine queues ----
        bias_t = bias_pool.tile([P, 3 * PAIRS], f32, name="bias", tag="bias")
        nc.sync.dma_start(bias_t[:], biases[:])
        bq_t = {p: bias_t[:, p:p + 1] for p in range(PAIRS)}
        bk_t = {p: bias_t[:, PAIRS + p:PAIRS + p + 1] for p in range(PAIRS)}
        bv_t = {p: bias_t[:, 2 * PAIRS + p:2 * PAIRS + p + 1] for p in range(PAIRS)}

        vctx = ExitStack()
        xv_pool = vctx.enter_context(tc.tile_pool(name="xv", bufs=FC))
        wv_pool = vctx.enter_context(tc.tile_pool(name="wvp", bufs=FC))
        pp = ctx.enter_context(tc.tile_pool(name="pp", bufs=2, space="PSUM"))

        wq_t, wk_t, wv_t = {}, {}, {}
        xq_t, xk_t, xv_t = [], [], []
        for f in range(FC):
            t = wqk_pool.tile([P, DH], bf16, name=f"wq_{f}", tag="wqk")
            nc.sync.dma_start(t[:], wq[f * P:(f + 1) * P, :])
            wq_t[f] = t
            t = wqk_pool.tile([P, DH], bf16, name=f"wk_{f}", tag="wqk")
            nc.scalar.dma_start(t[:], wk[f * P:(f + 1) * P, :])
            wk_t[f] = t
            t = wv_pool.tile([P, DH], bf16, name=f"wv_{f}", tag="wvp")
            nc.gpsimd.dma_start(t[:], wv[f * P:(f + 1) * P, :])
            wv_t[f] = t
        for f in range(FC):
            t = xq_pool.tile([P, S], bf16, name=f"xq_{f}", tag="xq")
            nc.sync.dma_start(t[:], xqT[f * P:(f + 1) * P, :])
            xq_t.append(t)
            t = xk_pool.tile([P, S], bf16, name=f"xk_{f}", tag="xk")
            nc.scalar.dma_start(t[:], xkT[f * P:(f + 1) * P, :])
            xk_t.append(t)
            t = xv_pool.tile([P, S], bf16, name=f"xv_{f}", tag="xv")
            nc.gpsimd.dma_start(t[:], xvT[f * P:(f + 1) * P, :])
            xv_t.append(t)
        wo_t = {}
        for p in range(PAIRS):
            for dc in range(2):
                t = wo_pool.tile([P, QW], bf16, name=f"wo_{p}_{dc}", tag="wop")
                nc.scalar.dma_start(t[:], wo[p * P:(p + 1) * P,
                                            dc * QW:(dc + 1) * QW])
                wo_t[(p, dc)] = t

        qt_t = [qt_pool.tile([P, S], bf16, name=f"qt_{p}", tag="qt")
                for p in range(PAIRS)]
        kt_t = [kt_pool.tile([P, S], bf16, name=f"kt_{p}", tag="kt")
                for p in range(PAIRS)]

        # ---- Q/K projection for one pair, one query-chunk quarter ----
        def qk_proj_quarter(p, tc4, pool):
            for (x_t, w_t, dst, b_t, nm) in ((xq_t, wq_t, qt_t, bq_t, "q"),
                                             (xk_t, wk_t, kt_t, bk_t, "k")):
                ps = pool.tile([P, QW], f32, name=f"ps{nm}_{p}_{tc4}", tag=pool.name)
                for f in range(FC):
                    mmr(ps[:], w_t[f][:, p * P:(p + 1) * P],
                        x_t[f][:, tc4 * QW:(tc4 + 1) * QW],
                        start=(f == 0), stop=(f == FC - 1))
                nc.vector.tensor_scalar_add(
                    dst[p][:, tc4 * QW:(tc4 + 1) * QW], ps[:], b_t[p][:])

        # ---- Phase A: Q/K projection for pair 0 ----
        for tc4 in range(QCC):
            qk_proj_quarter(0, tc4, pp)

        # ---- Phase B: V projection into resident V' tiles ----
        vpr_t = []
        for tci in range(TOK_T):
            ps = pp.tile([P, DH], f32, name=f"psv_{tci}", tag="pp")
            for f in range(FC):
                mmr(ps[:], xv_t[f][:, tci * P:(tci + 1) * P], wv_t[f][:],
                    start=(f == 0), stop=(f == FC - 1))
            vt = vpr_pool.tile([P, HPC * (HD + 1)], bf16,
                               name=f"vpr_{tci}", tag="vpr")
            v3 = vt.rearrange("p (h c) -> p h c", c=HD + 1)
            nc.gpsimd.memset(v3[:, :, HD:HD + 1], 1.0)
            nc.vector.tensor_copy(v3[:, :, 0:HD],
                                  ps.rearrange("p (h c) -> p h c", c=HD))
            vpr_t.append(vt)
        vctx.close()  # xv / wv SBUF freed for the E^T pool
        et_pool = ctx.enter_context(tc.tile_pool(name="et", bufs=10))
        pssc = ctx.enter_context(tc.tile_pool(name="pssc", bufs=4, space="PSUM"))
        pso = ctx.enter_context(tc.tile_pool(name="pso", bufs=2, space="PSUM"))

        # ---- attention unit: per-kc scores/exp with trailing PV ----
        # PV-A trails the score/exp stream by 2 key chunks, PV-B by 8; the
        # A-half normalize is emitted before the B tail so the DVE reciprocal
        # hides under PE work and frees poA for the next unit in time.
        def pv_one(p, qc, et, po, hh, kc):
            c0 = (2 * p + hh) * (HD + 1)
            half, koff = kc // 4, (kc % 4) * QW
            mmr(po[:], vpr_t[kc][:, c0:c0 + HD + 1],
                et[(hh, half)][:, koff:koff + QW],
                start=(kc == 0), stop=(kc == KC - 1))

        ATR, BTR = 2, 8

        def unit(p, qc, poA, poB):
            et = {}
            for hh in range(2):
                for quarter in range(4):
                    et[(hh, quarter)] = et_pool.tile(
                        [P, 4 * QW], bf16, name=f"et_{p}_{qc}_{hh}_{quarter}",
                        tag="et")
            for kc in range(KC):
                half, koff = kc // 4, (kc % 4) * QW
                psA = pssc.tile([P, QW], f32, name=f"scA_{p}_{qc}_{kc}", tag="pssc")
                psB = pssc.tile([P, QW], f32, name=f"scB_{p}_{qc}_{kc}", tag="pssc")
                mmr(psA[:], kt_t[p][0:64, kc * P:(kc + 1) * P],
                    qt_t[p][0:64, qc * QW:(qc + 1) * QW],
                    start=True, stop=True, tile_position=(0, 0))
                mmr(psB[:], kt_t[p][64:128, kc * P:(kc + 1) * P],
                    qt_t[p][64:128, qc * QW:(qc + 1) * QW],
                    start=True, stop=True, tile_position=(64, 0))
                nc.scalar.activation(et[(0, half)][:, koff:koff + QW],
                                     psA[:], AF.Exp, scale=SCALE)
                nc.scalar.activation(et[(1, half)][:, koff:koff + QW],
                                     psB[:], AF.Exp, scale=SCALE)
                if kc >= ATR:
                    pv_one(p, qc, et, poA, 0, kc - ATR)
                if kc >= BTR:
                    pv_one(p, qc, et, poB, 1, kc - BTR)
            ot_t = ot_pool.tile([P, QW], bf16, name=f"ot_{p}_{qc}", tag="ot")
            for kc in range(KC - ATR, KC):
                pv_one(p, qc, et, poA, 0, kc)
            normalize_half(p, qc, poA, 0, ot_t)
            for kc in range(KC - BTR, KC):
                pv_one(p, qc, et, poB, 1, kc)
            normalize_half(p, qc, poB, 1, ot_t)
            nc.vector.tensor_scalar_add(ot_t[:], ot_t[:], bv_t[p][:])
            return ot_t

        def normalize_half(p, qc, po, hh, ot_t):
            zr = zr_pool.tile([1, QW], f32, name=f"zr_{p}_{qc}_{hh}", tag="zr")
            nc.vector.reciprocal(zr[:], po[64:65, :])
            zb = zb_pool.tile([64, QW], f32, name=f"zb_{p}_{qc}_{hh}", tag="zb")
            nc.gpsimd.partition_broadcast(zb[:], zr[:])
            nc.vector.tensor_mul(ot_t[hh * 64:(hh + 1) * 64, :],
                                 po[0:64, :], zb[:])

        def outproj(qc, ots):
            for tl in range(QW // P):
                tci = qc * (QW // P) + tl
                for dc in range(2):
                    ps = pp.tile([P, QW], f32, name=f"pout_{tci}_{dc}", tag="pp")
                    for pq in range(PAIRS):
                        mmr(ps[:], ots[pq][:, tl * P:(tl + 1) * P],
                            wo_t[(pq, dc)][:],
                            start=(pq == 0), stop=(pq == PAIRS - 1))
                    ost = os_pool.tile([P, QW], f32,
                                       name=f"os_{tci}_{dc}", tag="os")
                    nc.vector.tensor_copy(ost[:], ps[:])
                    nc.sync.dma_start(out[tci * P:(tci + 1) * P,
                                          dc * QW:(dc + 1) * QW], ost[:])

        # ---- Phase C: pair-outer sweep, next pair's Q/K proj interleaved ----
        ots_by_qc = {qc: [None] * PAIRS for qc in range(QCC)}
        for p in range(PAIRS):
            for qc in range(QCC):
                poA = pso.tile([HD + 1, QW], f32, name=f"poA_{p}_{qc}", tag="pso")
                poB = pso.tile([HD + 1, QW], f32, name=f"poB_{p}_{qc}", tag="pso")
                ots_by_qc[qc][p] = unit(p, qc, poA, poB)
                if p < PAIRS - 1:
                    qk_proj_quarter(p + 1, qc, pp)
                else:
                    outproj(qc, ots_by_qc[qc])
    nc.compile()
    return nc


def _get_nc():
    if "nc" not in _CACHE:
        _CACHE["nc"] = _build()
    return _CACHE["nc"]


def _in_maps(inputs):
    f = np.float32
    bf = ml_dtypes.bfloat16
    maps = []
    for c in range(NCORES):
        b, g = c // 2, c % 2
        hs = slice(g * HPC, (g + 1) * HPC)
        maps.append({
            "xqT": np.asarray(inputs["inputs_q"][b], f).T.astype(bf),
            "xkT": np.asarray(inputs["inputs_k"][b], f).T.astype(bf),
            "xvT": np.asarray(inputs["inputs_v"][b], f).T.astype(bf),
            "wq": np.asarray(inputs["Wq"], f)[:, hs, :].reshape(D, DH).astype(bf),
            "wk": np.asarray(inputs["Wk"], f)[:, hs, :].reshape(D, DH).astype(bf),
            "wv": np.asarray(inputs["Wv"], f)[:, hs, :].reshape(D, DH).astype(bf),
            "wo": np.asarray(inputs["Wo"], f)[hs].reshape(DH, D).astype(bf),
            "biases": np.stack(
                [np.asarray(inputs[nm], f)[hs].reshape(DH)[p * P:(p + 1) * P]
                 for nm in ("bq", "bk", "bv") for p in range(PAIRS)],
                axis=1).copy(),
        })
    return maps


def run_sharded(inputs, **kw):
    """Compile/run on all 8 cores; returns (full_output, BassKernelResults)."""
    from concourse.bass_utils import run_bass_kernel_spmd
    nc = _get_nc()
    res = run_bass_kernel_spmd(nc, _in_maps(inputs), core_ids=list(range(NCORES)), **kw)
    bo = np.asarray(inputs["bo"], np.float32)
    full = np.empty((B, S, D), np.float32)
    for b in range(B):
        full[b] = res.results[2 * b]["out"] + res.results[2 * b + 1]["out"] + bo
    return full, res


def kernel(**inputs) -> np.ndarray:
    full, _ = run_sharded(inputs)
    return full


# revision 13
# speedup vs baseline: 1.2819x; 1.2819x over previous
"""Trainium2 Bass kernel for nn_MultiHeadAttention (B=4, S=2048, D=1024, H=16, HD=64).

Sharding: 8 cores = 4 batches (data parallel) x 2 head-groups of 8 heads
(tensor parallel). Each core computes its batch's QKV projections for its 8
heads, full softmax attention, and the partial output projection for its head
group. The host sums the two head-group partials per batch (the hinted
all-reduce, done at gather time) and adds the output bias.

v2 vs baseline: all matmul operands in bf16 (the fp32r sweep tripped the HAM
power throttle to a 50% duty cycle; bf16 also halves DMA+SBUF), approximate
reciprocal for the softmax denominator (exact DVE reciprocal was 3.3us per
call and gated each unit), and a pair-outer sweep with the Q/K projections of
pair p+1 interleaved between attention units of pair p so the ACT engine
(which carries all 33.5M exp elements -- the ~255us floor) starts ~40us
earlier and never starves.

Per-core layout (all matmuls bf16, N=512 -> 1 cycle/row):
  - x^T tiles resident in SBUF; Q^T/K^T [dh, tok] per head pair, scores
    transposed S^T[k, q] via PE row packing (tile_position (0,0)/(64,0), K=64).
  - exp on ACT from 2-bank PSUM [128, 1024] into E^T bf16 tiles (scale folded).
  - PV: lhsT = V' [128, 65] with a ones column collecting the denominator Z.
  - normalize: DVE reciprocal_approx_fast on Z + gpsimd partition_broadcast +
    DVE multiply -> O^T bf16; out-proj per qc after pair 3's unit.
PSUM budget: pp 2x[128,512] + pssc 2x[128,1024] + pso 2x[65,512] = 8 banks.
"""

import numpy as np
import ml_dtypes
from contextlib import ExitStack

B, S, D = 4, 2048, 1024
H, HD = 16, 64
NCORES = 8
HPC = H // 2            # heads per core = 8
PAIRS = HPC // 2        # head pairs per core = 4
DH = HPC * HD           # per-core head dims = 512
P = 128
TOK_T = S // P          # 16 token tiles of 128
QCC = S // 512          # 4 query chunks of 512
QW = 512
KC = S // P             # 16 key chunks of 128
FC = D // P             # 8 feature chunks of 128
KPG = 2                 # key tiles per exp group
NG = KC // KPG          # 8 exp groups per (pair, qc)

_CACHE = {}


def _build():
    import concourse.bacc as bacc
    import concourse.mybir as mybir
    import concourse.tile as tile

    dt = mybir.dt
    f32 = dt.float32
    bf16 = dt.bfloat16
    AF = mybir.ActivationFunctionType

    nc = bacc.Bacc("TRN2", target_bir_lowering=False, debug=False)

    xqT = nc.dram_tensor("xqT", [D, S], bf16, kind="ExternalInput")
    xkT = nc.dram_tensor("xkT", [D, S], bf16, kind="ExternalInput")
    xvT = nc.dram_tensor("xvT", [D, S], bf16, kind="ExternalInput")
    wq = nc.dram_tensor("wq", [D, DH], bf16, kind="ExternalInput")
    wk = nc.dram_tensor("wk", [D, DH], bf16, kind="ExternalInput")
    wv = nc.dram_tensor("wv", [D, DH], bf16, kind="ExternalInput")
    wo = nc.dram_tensor("wo", [DH, D], bf16, kind="ExternalInput")
    biases = nc.dram_tensor("biases", [P, 3 * PAIRS], f32, kind="ExternalInput")
    out = nc.dram_tensor("out", [S, D], f32, kind="ExternalOutput")

    SCALE = 1.0 / float(np.sqrt(HD))

    def mmr(psum, lhsT, rhs, **kw):
        nc.tensor.matmul(psum, lhsT, rhs, **kw)

    with tile.TileContext(nc, pool_alloc_mode="queue") as tc, ExitStack() as ctx:
        # ---- pools ----
        xq_pool = ctx.enter_context(tc.tile_pool(name="xq", bufs=FC))
        xk_pool = ctx.enter_context(tc.tile_pool(name="xk", bufs=FC))
        wqk_pool = ctx.enter_context(tc.tile_pool(name="wqk", bufs=2 * FC))
        wo_pool = ctx.enter_context(tc.tile_pool(name="wop", bufs=2 * PAIRS))
        qt_pool = ctx.enter_context(tc.tile_pool(name="qt", bufs=PAIRS))
        kt_pool = ctx.enter_context(tc.tile_pool(name="kt", bufs=PAIRS))
        vpr_pool = ctx.enter_context(tc.tile_pool(name="vpr", bufs=TOK_T))
        ot_pool = ctx.enter_context(tc.tile_pool(name="ot", bufs=16))
        zr_pool = ctx.enter_context(tc.tile_pool(name="zr", bufs=2))
        zb_pool = ctx.enter_context(tc.tile_pool(name="zb", bufs=2))
        os_pool = ctx.enter_context(tc.tile_pool(name="os", bufs=2))
        bias_pool = ctx.enter_context(tc.tile_pool(name="bias", bufs=1))

        # ---- DMA: spread input loads across engine queues ----
        bias_t = bias_pool.tile([P, 3 * PAIRS], f32, name="bias", tag="bias")
        nc.sync.dma_start(bias_t[:], biases[:])
        bq_t = {p: bias_t[:, p:p + 1] for p in range(PAIRS)}
        bk_t = {p: bias_t[:, PAIRS + p:PAIRS + p + 1] for p in range(PAIRS)}
        bv_t = {p: bias_t[:, 2 * PAIRS + p:2 * PAIRS + p + 1] for p in range(PAIRS)}

        vctx = ExitStack()
        xv_pool = vctx.enter_context(tc.tile_pool(name="xv", bufs=FC))
        wv_pool = vctx.enter_context(tc.tile_pool(name="wvp", bufs=FC))
        pp = ctx.enter_context(tc.tile_pool(name="pp", bufs=2, space="PSUM"))

        wq_t, wk_t, wv_t = {}, {}, {}
        xq_t, xk_t, xv_t = [], [], []
        for f in range(FC):
            t = wqk_pool.tile([P, DH], bf16, name=f"wq_{f}", tag="wqk")
            nc.sync.dma_start(t[:], wq[f * P:(f + 1) * P, :])
            wq_t[f] = t
            t = wqk_pool.tile([P, DH], bf16, name=f"wk_{f}", tag="wqk")
            nc.scalar.dma_start(t[:], wk[f * P:(f + 1) * P, :])
            wk_t[f] = t
            t = wv_pool.tile([P, DH], bf16, name=f"wv_{f}", tag="wvp")
            nc.gpsimd.dma_start(t[:], wv[f * P:(f + 1) * P, :])
            wv_t[f] = t
        for f in range(FC):
            t = xq_pool.tile([P, S], bf16, name=f"xq_{f}", tag="xq")
            nc.sync.dma_start(t[:], xqT[f * P:(f + 1) * P, :])
            xq_t.append(t)
            t = xk_pool.tile([P, S], bf16, name=f"xk_{f}", tag="xk")
            nc.scalar.dma_start(t[:], xkT[f * P:(f + 1) * P, :])
            xk_t.append(t)
            t = xv_pool.tile([P, S], bf16, name=f"xv_{f}", tag="xv")
            nc.gpsimd.dma_start(t[:], xvT[f * P:(f + 1) * P, :])
            xv_t.append(t)
        wo_t = {}
        for p in range(PAIRS):
            for dc in range(2):
                t = wo_pool.tile([P, QW], bf16, name=f"wo_{p}_{dc}", tag="wop")
                nc.scalar.dma_start(t[:], wo[p * P:(p + 1) * P,
                                            dc * QW:(dc + 1) * QW])
                wo_t[(p, dc)] = t

        qt_t = [qt_pool.tile([P, S], bf16, name=f"qt_{p}", tag="qt")
                for p in range(PAIRS)]
        kt_t = [kt_pool.tile([P, S], bf16, name=f"kt_{p}", tag="kt")
                for p in range(PAIRS)]

        # ---- Q/K projection for one pair, one query-chunk quarter ----
        def qk_proj_quarter(p, tc4, pool):
            for (x_t, w_t, dst, b_t, nm) in ((xq_t, wq_t, qt_t, bq_t, "q"),
                                             (xk_t, wk_t, kt_t, bk_t, "k")):
                ps = pool.tile([P, QW], f32, name=f"ps{nm}_{p}_{tc4}", tag=pool.name)
                for f in range(FC):
                    mmr(ps[:], w_t[f][:, p * P:(p + 1) * P],
                        x_t[f][:, tc4 * QW:(tc4 + 1) * QW],
                        start=(f == 0), stop=(f == FC - 1))
                nc.vector.tensor_scalar_add(
                    dst[p][:, tc4 * QW:(tc4 + 1) * QW], ps[:], b_t[p][:])

        # ---- Phase A: Q/K projection for pair 0 ----
        for tc4 in range(QCC):
            qk_proj_quarter(0, tc4, pp)

        # ---- Phase B: V projection into resident V' tiles ----
        vpr_t = []
        for tci in range(TOK_T):
            ps = pp.tile([P, DH], f32, name=f"psv_{tci}", tag="pp")
            for f in range(FC):
                mmr(ps[:], xv_t[f][:, tci * P:(tci + 1) * P], wv_t[f][:],
                    start=(f == 0), stop=(f == FC - 1))
            vt = vpr_pool.tile([P, HPC * (HD + 1)], bf16,
                               name=f"vpr_{tci}", tag="vpr")
            v3 = vt.rearrange("p (h c) -> p h c", c=HD + 1)
            nc.gpsimd.memset(v3[:, :, HD:HD + 1], 1.0)
            nc.vector.tensor_copy(v3[:, :, 0:HD],
                                  ps.rearrange("p (h c) -> p h c", c=HD))
            vpr_t.append(vt)
        vctx.close()  # xv / wv SBUF freed for the E^T pool
        et_pool = ctx.enter_context(tc.tile_pool(name="et", bufs=10))
        pssc = ctx.enter_context(tc.tile_pool(name="pssc", bufs=2, space="PSUM"))
        pso = ctx.enter_context(tc.tile_pool(name="pso", bufs=2, space="PSUM"))

        # ---- attention unit (v2 structure): scores+exp groups, then PV ----
        def unit(p, qc, poA, poB):
            et = {}
            for hh in range(2):
                for quarter in range(4):
                    et[(hh, quarter)] = et_pool.tile(
                        [P, 4 * QW], bf16, name=f"et_{p}_{qc}_{hh}_{quarter}",
                        tag="et")
            for g in range(NG):
                half, goff = g // 2, (g % 2) * KPG * QW
                psA = pssc.tile([P, KPG * QW], f32,
                                name=f"scA_{p}_{qc}_{g}", tag="pssc")
                psB = pssc.tile([P, KPG * QW], f32,
                                name=f"scB_{p}_{qc}_{g}", tag="pssc")
                for j in range(KPG):
                    kc = g * KPG + j
                    mmr(psA[:, j * QW:(j + 1) * QW],
                        kt_t[p][0:64, kc * P:(kc + 1) * P],
                        qt_t[p][0:64, qc * QW:(qc + 1) * QW],
                        start=True, stop=True, tile_position=(0, 0))
                    mmr(psB[:, j * QW:(j + 1) * QW],
                        kt_t[p][64:128, kc * P:(kc + 1) * P],
                        qt_t[p][64:128, qc * QW:(qc + 1) * QW],
                        start=True, stop=True, tile_position=(64, 0))
                nc.scalar.activation(et[(0, half)][:, goff:goff + KPG * QW],
                                     psA[:], AF.Exp, scale=SCALE)
                nc.scalar.activation(et[(1, half)][:, goff:goff + KPG * QW],
                                     psB[:], AF.Exp, scale=SCALE)
            for kc in range(KC):
                half, koff = kc // 4, (kc % 4) * QW
                cA = (2 * p) * (HD + 1)
                cB = (2 * p + 1) * (HD + 1)
                mmr(poA[:], vpr_t[kc][:, cA:cA + HD + 1],
                    et[(0, half)][:, koff:koff + QW],
                    start=(kc == 0), stop=(kc == KC - 1))
                mmr(poB[:], vpr_t[kc][:, cB:cB + HD + 1],
                    et[(1, half)][:, koff:koff + QW],
                    start=(kc == 0), stop=(kc == KC - 1))
            ot_t = ot_pool.tile([P, QW], bf16, name=f"ot_{p}_{qc}", tag="ot")
            normalize_half(p, qc, poA, 0, ot_t)
            normalize_half(p, qc, poB, 1, ot_t)
            nc.vector.tensor_scalar_add(ot_t[:], ot_t[:], bv_t[p][:])
            return ot_t

        # 1/Z on ACT via Ln then Exp(-x): both live in the
        # natural_log_exp_and_others table set, so no table reloads against
        # the softmax Exp. The exact DVE reciprocal (3.3us) sat on the pso
        # critical path; this chain is ~1.2us and runs in the ACT boundary
        # idle between units.
        def normalize_half(p, qc, po, hh, ot_t):
            zl = zr_pool.tile([1, QW], f32, name=f"zl_{p}_{qc}_{hh}", tag="zr")
            nc.scalar.activation(zl[:], po[64:65, :], AF.Ln)
            zr = zr_pool.tile([1, QW], f32, name=f"zr_{p}_{qc}_{hh}", tag="zr")
            nc.scalar.activation(zr[:], zl[:], AF.Exp, scale=-1.0)
            zb = zb_pool.tile([64, QW], f32, name=f"zb_{p}_{qc}_{hh}", tag="zb")
            nc.gpsimd.partition_broadcast(zb[:], zr[:])
            nc.vector.tensor_mul(ot_t[hh * 64:(hh + 1) * 64, :],
                                 po[0:64, :], zb[:])

        def outproj(qc, ots):
            for tl in range(QW // P):
                tci = qc * (QW // P) + tl
                for dc in range(2):
                    ps = pp.tile([P, QW], f32, name=f"pout_{tci}_{dc}", tag="pp")
                    for pq in range(PAIRS):
                        mmr(ps[:], ots[pq][:, tl * P:(tl + 1) * P],
                            wo_t[(pq, dc)][:],
                            start=(pq == 0), stop=(pq == PAIRS - 1))
                    ost = os_pool.tile([P, QW], f32,
                                       name=f"os_{tci}_{dc}", tag="os")
                    nc.vector.tensor_copy(ost[:], ps[:])
                    nc.sync.dma_start(out[tci * P:(tci + 1) * P,
                                          dc * QW:(dc + 1) * QW], ost[:])

        # ---- Phase C: pair-outer sweep, next pair's Q/K proj interleaved ----
        ots_by_qc = {qc: [None] * PAIRS for qc in range(QCC)}
        for p in range(PAIRS):
            for qc in range(QCC):
                poA = pso.tile([HD + 1, QW], f32, name=f"poA_{p}_{qc}", tag="pso")
                poB = pso.tile([HD + 1, QW], f32, name=f"poB_{p}_{qc}", tag="pso")
                ots_by_qc[qc][p] = unit(p, qc, poA, poB)
                if p < PAIRS - 1:
                    qk_proj_quarter(p + 1, qc, pp)
                else:
                    outproj(qc, ots_by_qc[qc])
    nc.compile()
    return nc


def _get_nc():
    if "nc" not in _CACHE:
        _CACHE["nc"] = _build()
    return _CACHE["nc"]


def _in_maps(inputs):
    f = np.float32
    bf = ml_dtypes.bfloat16
    maps = []
    for c in range(NCORES):
        b, g = c // 2, c % 2
        hs = slice(g * HPC, (g + 1) * HPC)
        maps.append({
            "xqT": np.asarray(inputs["inputs_q"][b], f).T.astype(bf),
            "xkT": np.asarray(inputs["inputs_k"][b], f).T.astype(bf),
            "xvT": np.asarray(inputs["inputs_v"][b], f).T.astype(bf),
            "wq": np.asarray(inputs["Wq"], f)[:, hs, :].reshape(D, DH).astype(bf),
            "wk": np.asarray(inputs["Wk"], f)[:, hs, :].reshape(D, DH).astype(bf),
            "wv": np.asarray(inputs["Wv"], f)[:, hs, :].reshape(D, DH).astype(bf),
            "wo": np.asarray(inputs["Wo"], f)[hs].reshape(DH, D).astype(bf),
            "biases": np.stack(
                [np.asarray(inputs[nm], f)[hs].reshape(DH)[p * P:(p + 1) * P]
                 for nm in ("bq", "bk", "bv") for p in range(PAIRS)],
                axis=1).copy(),
        })
    return maps


def run_sharded(inputs, **kw):
    """Compile/run on all 8 cores; returns (full_output, BassKernelResults)."""
    from concourse.bass_utils import run_bass_kernel_spmd
    nc = _get_nc()
    res = run_bass_kernel_spmd(nc, _in_maps(inputs), core_ids=list(range(NCORES)), **kw)
    bo = np.asarray(inputs["bo"], np.float32)
    full = np.empty((B, S, D), np.float32)
    for b in range(B):
        full[b] = res.results[2 * b]["out"] + res.results[2 * b + 1]["out"] + bo
    return full, res


def kernel(**inputs) -> np.ndarray:
    full, _ = run_sharded(inputs)
    return full
